# revision 26
# baseline (speedup 1.0000x reference)
"""Trainium2 Bass kernel for nn_BondDecoder (histogram_binning).

Math (per batch element b, all derived exactly from the reference):
  a_i = 1 - src_mask ; t_i = tgt_mask ; g = 1 - t ; c = a*t
  pad_mask*or_mask = a_i a_j - c_i c_j      (both 0/1 masks)
  4*(inc-dec)      = sum_h softmax_inc_h - sum_h softmax_dec_h
  loss_b = sum_ij (a_i a_j - c_i c_j) * z_ij^2
  z = sum_h softmax_inc_h - sum_h softmax_dec_h + H_src - (g_i g_j) H_tgt

Device pipeline per core (4 batch elements):
  - fp16 matmuls on PE: folded projections (conv1d+inproj fused on host),
    per-head QK^T scores, key-mask folded in as a rank-1 PSUM accumulate
    (-30000 * mask broadcast over rows), final masked reduction as two
    quadratic forms  a^T (z*z) a  and  c^T (z*z) c.
  - ACT: PSUM->SBUF copies, Exp with accum_out giving masked row sums.
  - DVE: reciprocal, fused normalize+head-sum+D-add via chained
    scalar_tensor_tensor, z^2, final dot+reduce.
K-side in-projection bias is dropped: it shifts every score in a softmax
row by a constant, which cancels exactly in softmax.
"""

from contextlib import ExitStack

import numpy as np

import concourse.bacc as bacc
import concourse.mybir as mybir
import concourse.tile as tile
from concourse.bass_utils import run_bass_kernel_spmd

L = 512
B = 32
D = 512
NCORES = 8
BPC = B // NCORES  # batch elements per core
NH = 4
HD = D // NH  # 128
NI = L // 128  # i-chunks per batch element
SCALE = float(1.0 / np.sqrt(HD))
MASKNEG = -30000.0

F16 = mybir.dt.float16
F32 = mybir.dt.float32
AF = mybir.ActivationFunctionType
ALU = mybir.AluOpType

_CACHE = {}


def _emit(ctx, tc, dram, out_ap, repeat=1):
    nc = tc.nc

    const_pool = ctx.enter_context(tc.tile_pool(name="const", bufs=1))
    xt_pool = ctx.enter_context(tc.tile_pool(name="xt", bufs=2))
    qk_pool = ctx.enter_context(tc.tile_pool(name="qk", bufs=3))
    e_pool = ctx.enter_context(tc.tile_pool(name="e", bufs=3))
    z_pool = ctx.enter_context(tc.tile_pool(name="z", bufs=3))
    small_pool = ctx.enter_context(tc.tile_pool(name="small", bufs=3))
    psum_proj = ctx.enter_context(tc.tile_pool(name="pproj", bufs=2, space="PSUM"))
    psum_s = ctx.enter_context(tc.tile_pool(name="pscore", bufs=5, space="PSUM"))
    psum_q = ctx.enter_context(tc.tile_pool(name="pquad", bufs=1, space="PSUM"))

    # constants / parameters
    acat_t = []
    for e in range(4):
        t = const_pool.tile([128, 4 * D], F16, tag=f"acat{e}")
        nc.sync.dma_start(t[:], dram["acat"][128 * e : 128 * (e + 1), :])
        acat_t.append(t)
    qbr_t = const_pool.tile([128, 16], F32, tag="qbr")
    nc.sync.dma_start(qbr_t[:], dram["qbr"][:])
    ones_col = const_pool.tile([1, 128], F16, tag="ones_col")
    nc.gpsimd.memset(ones_col[:], 1.0)

    for b in [b for _ in range(repeat) for b in range(BPC)]:
        # ---- load x^T for this batch element ----
        xt_t = []
        for e in range(4):
            t = xt_pool.tile([128, L], F16, tag=f"xt{e}")
            nc.sync.dma_start(t[:], dram["xt"][b, 128 * e : 128 * (e + 1), :])
            xt_t.append(t)
        mneg_t = small_pool.tile([1, L], F16, tag="mneg")
        nc.sync.dma_start(mneg_t[:], dram["mneg"][b])
        acr_t = small_pool.tile([2, L], F32, tag="acr")
        nc.sync.dma_start(acr_t[:], dram["acr"][b])
        ac_t = small_pool.tile([128, 8], F16, tag="ac")
        nc.sync.dma_start(ac_t[:], dram["acb"][b])
        d_ts = []
        for ic in range(4):
            d_t = z_pool.tile([128, L], F16, tag=f"d{ic}")
            nc.sync.dma_start(d_t[:], dram["dmat"][b, 128 * ic : 128 * (ic + 1), :])
            d_ts.append(d_t)

        # ---- projections: QK^T = Acat^T @ x^T ----
        # chain layout along the 2048 columns: [q_inc | k_inc | q_dec | k_dec]
        qk = []
        for dc in range(16):
            ps = psum_proj.tile([128, L], F32, tag="pproj")
            for e in range(4):
                nc.tensor.matmul(
                    ps[:],
                    acat_t[e][:, 128 * dc : 128 * (dc + 1)],
                    xt_t[e][:],
                    start=(e == 0),
                    stop=(e == 3),
                )
            t = qk_pool.tile([128, L], F16, tag=f"qk{dc}")
            # q chains: add the folded bias on the copy; k chains: bias
            # cancels in softmax, zero bias column makes it a plain copy
            nc.vector.tensor_scalar_add(t[:], ps[:], qbr_t[:, dc : dc + 1])
            qk.append(t)

        # ---- scores, masked exp, normalize, head-sum, +D, square ----
        zsq = []
        for ic in range(4):
            rs = small_pool.tile([128, 8], F32, tag="rs")
            E = []
            for attn in range(2):
                for h in range(NH):
                    qdc = (0 if attn == 0 else 8) + h
                    kdc = (4 if attn == 0 else 12) + h
                    ps = psum_s.tile([128, L], F32, tag="pscore")
                    nc.tensor.matmul(
                        ps[:],
                        qk[qdc][:, 128 * ic : 128 * (ic + 1)],
                        qk[kdc][:],
                        start=True,
                        stop=False,
                    )
                    nc.tensor.matmul(
                        ps[:], ones_col[:], mneg_t[:], start=False, stop=True
                    )
                    g = attn * 4 + h
                    e_t = e_pool.tile([128, L], F16, tag=f"e{g}")
                    nc.scalar.activation(
                        e_t[:],
                        ps[:],
                        AF.Exp,
                        scale=SCALE,
                        accum_out=rs[:, g : g + 1],
                    )
                    E.append(e_t)
            w = small_pool.tile([128, 8], F32, tag="w")
            nc.vector.reciprocal(w[:], rs[:])
            wn = small_pool.tile([128, 4], F32, tag="wn")
            nc.vector.tensor_scalar_mul(wn[:], w[:, 4:8], -1.0)

            # normalize each head with 4x-mode tensor_scalar, then sum via
            # 2x-mode tensor_tensor adds, seeded with the D tile (the chain
            # absorbs the histogram term). Faster than 1x scalar_tensor_tensor.
            en = []
            for g in range(8):
                wg = w[:, g : g + 1] if g < 4 else wn[:, g - 4 : g - 3]
                t = e_pool.tile([128, L], F16, tag=f"en{g}")
                nc.vector.tensor_scalar_mul(t[:], E[g][:], wg)
                en.append(t)
            acc = d_ts[ic]
            for g in range(8):
                nxt = z_pool.tile([128, L], F16, tag=f"zacc{g % 2}")
                nc.vector.tensor_add(nxt[:], en[g][:], acc[:])
                acc = nxt
            zq = z_pool.tile([128, L], F16, tag="zsq")
            nc.scalar.activation(zq[:], acc[:], AF.Square)
            zsq.append(zq)

        # ---- quadratic forms: rows [a^T W ; c^T W], W = z*z ----
        qf = psum_q.tile([2, L], F32, tag="qf")
        for ic in range(4):
            nc.tensor.matmul(
                qf[:],
                ac_t[:, 2 * ic : 2 * (ic + 1)],
                zsq[ic][:],
                start=(ic == 0),
                stop=(ic == 3),
            )
        # ---- final dots: sum_j (a^T W)_j a_j  and  -sum_j (c^T W)_j c_j ----
        fd = small_pool.tile([2, L], F32, tag="fd")
        red = small_pool.tile([2, 1], F32, tag="red")
        nc.vector.tensor_mul(fd[:], qf[:], acr_t[:])
        nc.vector.tensor_reduce(red[:], fd[:], axis=mybir.AxisListType.X, op=ALU.add)
        nc.sync.dma_start(out_ap[b], red[:])


def _build(repeat=1):
    nc = bacc.Bacc(
        "TRN2",
        target_bir_lowering=False,
        debug=False,
        num_devices=NCORES,
    )
    dram = {
        "acat": nc.dram_tensor("acat", [D, 4 * D], F16, kind="ExternalInput").ap(),
        "qbr": nc.dram_tensor("qbr", [128, 16], F32, kind="ExternalInput").ap(),
        "xt": nc.dram_tensor("xt", [BPC, D, L], F16, kind="ExternalInput").ap(),
        "mneg": nc.dram_tensor("mneg", [BPC, 1, L], F16, kind="ExternalInput").ap(),
        "dmat": nc.dram_tensor("dmat", [BPC, L, L], F16, kind="ExternalInput").ap(),
        "acb": nc.dram_tensor("acb", [BPC, 128, 8], F16, kind="ExternalInput").ap(),
        "acr": nc.dram_tensor("acr", [BPC, 2, L], F32, kind="ExternalInput").ap(),
    }
    out_ap = nc.dram_tensor("out", [BPC, 2], F32, kind="ExternalOutput").ap()
    with tile.TileContext(nc) as tc, ExitStack() as ctx:
        _emit(ctx, tc, dram, out_ap, repeat=repeat)
    nc.compile()
    return nc


def get_nc(repeat=1):
    key = f"nc{repeat}"
    if key not in _CACHE:
        _CACHE[key] = _build(repeat=repeat)
    return _CACHE[key]


def _fold(cw, cb, W, bb):
    # q = (x @ cw.T + cb) @ W.T + bb  ==  x @ A + bias
    A = (W.astype(np.float64) @ cw.astype(np.float64)).T
    bias = cb.astype(np.float64) @ W.astype(np.float64).T + bb
    return A.astype(np.float32), bias.astype(np.float32)


def prepare_in_maps(inputs):
    me = np.asarray(inputs["molecule_embedding"], np.float32)  # [L, B, D]
    src_bond = np.asarray(inputs["src_bond"]).astype(np.int64)  # [B, L, 6]
    tgt_bond = np.asarray(inputs["tgt_bond"]).astype(np.int64)
    src_mask = np.asarray(inputs["src_mask"]).astype(bool)  # [B, L]
    tgt_mask = np.asarray(inputs["tgt_mask"]).astype(bool)

    A_qi, b_qi = _fold(inputs["inc_q_w"], inputs["inc_q_b"], inputs["inc_Wq"], inputs["inc_bq"])
    A_ki, _ = _fold(inputs["inc_k_w"], inputs["inc_k_b"], inputs["inc_Wk"], inputs["inc_bk"])
    A_qd, b_qd = _fold(inputs["dec_q_w"], inputs["dec_q_b"], inputs["dec_Wq"], inputs["dec_bq"])
    A_kd, _ = _fold(inputs["dec_k_w"], inputs["dec_k_b"], inputs["dec_Wk"], inputs["dec_bk"])
    acat = np.concatenate([A_qi, A_ki, A_qd, A_kd], axis=1).astype(np.float16)
    # [128, 16] fp32: bias for d-chunk dc lives in column dc (zeros for k chains)
    qbr = (
        np.concatenate([b_qi, np.zeros(D, np.float32), b_qd, np.zeros(D, np.float32)])
        .astype(np.float32)
        .reshape(16, 128)
        .T.copy()
    )

    a = 1.0 - src_mask.astype(np.float32)  # pad
    t = tgt_mask.astype(np.float32)
    g = 1.0 - t
    c = a * t

    # bond histograms -> D = H_src - (g_i g_j) H_tgt   (small exact integers)
    bi = np.arange(B)[:, None, None]
    li = np.arange(L)[None, :, None]
    H_s = np.zeros((B, L, L), np.float32)
    np.add.at(H_s, (bi, li, src_bond), 1.0)
    H_t = np.zeros((B, L, L), np.float32)
    np.add.at(H_t, (bi, li, tgt_bond), 1.0)
    Dm = (H_s - g[:, :, None] * g[:, None, :] * H_t).astype(np.float16)

    mneg = (MASKNEG * src_mask.astype(np.float32)).astype(np.float16)[:, None, :]
    # [B, 128, 8]: row p, cols [2*ic, 2*ic+1] = (a, c) at token ic*128+p
    acb = (
        np.stack([a, c], axis=-1)  # [B, L, 2]
        .reshape(B, 4, 128, 2)
        .transpose(0, 2, 1, 3)
        .reshape(B, 128, 8)
        .astype(np.float16)
    )
    acr = np.stack([a, -c], axis=1).astype(np.float32)  # [B, 2, L]
    xt = np.ascontiguousarray(me.transpose(1, 2, 0)).astype(np.float16)  # [B, D, L]

    in_maps = []
    for cid in range(NCORES):
        sl = slice(cid * BPC, (cid + 1) * BPC)
        in_maps.append(
            {
                "acat": acat,
                "qbr": qbr,
                "xt": np.ascontiguousarray(xt[sl]),
                "mneg": np.ascontiguousarray(mneg[sl]),
                "dmat": np.ascontiguousarray(Dm[sl]),
                "acb": np.ascontiguousarray(acb[sl]),
                "acr": np.ascontiguousarray(acr[sl]),
            }
        )
    return in_maps


def finish(results):
    outp = np.concatenate([r["out"] for r in results], axis=0)  # [B, 2]
    return (outp[:, 0] + outp[:, 1]).astype(np.float32)


def kernel(**inputs):
    in_maps = prepare_in_maps(inputs)
    nc = get_nc()
    res = run_bass_kernel_spmd(nc, in_maps, core_ids=list(range(NCORES)))
    return finish(res.results)


if __name__ == "__main__":
    rng = np.random.default_rng(0)
    demo = {"molecule_embedding": rng.standard_normal((L, B, D), dtype=np.float32)}
    print("kernel module loaded OK")


# revision 27
# speedup vs baseline: 6.1257x; 6.1257x over previous
"""Trainium2 Bass kernel for nn_BondDecoder (histogram_binning).

Math (per batch element b, all derived exactly from the reference):
  a_i = 1 - src_mask ; t_i = tgt_mask ; g = 1 - t ; c = a*t
  pad_mask*or_mask = a_i a_j - c_i c_j      (both 0/1 masks)
  4*(inc-dec)      = sum_h softmax_inc_h - sum_h softmax_dec_h
  loss_b = sum_ij (a_i a_j - c_i c_j) * z_ij^2
  z = sum_h softmax_inc_h - sum_h softmax_dec_h + H_src - (g_i g_j) H_tgt

Device pipeline per core (4 batch elements):
  - fp16 matmuls on PE: folded projections (conv1d+inproj fused on host),
    per-head QK^T scores, key-mask folded in as a rank-1 PSUM accumulate
    (-30000 * mask broadcast over rows), final masked reduction as two
    quadratic forms  a^T (z*z) a  and  c^T (z*z) c.
  - ACT: PSUM->SBUF copies, Exp with accum_out giving masked row sums.
  - DVE: reciprocal, fused normalize+head-sum+D-add via chained
    scalar_tensor_tensor, z^2, final dot+reduce.
K-side in-projection bias is dropped: it shifts every score in a softmax
row by a constant, which cancels exactly in softmax.
"""

from contextlib import ExitStack

import numpy as np

import concourse.bacc as bacc
import concourse.mybir as mybir
import concourse.tile as tile
from concourse.bass_utils import run_bass_kernel_spmd

L = 512
B = 32
D = 512
NCORES = 8
BPC = B // NCORES  # batch elements per core
NH = 4
HD = D // NH  # 128
NI = L // 128  # i-chunks per batch element
SCALE = float(1.0 / np.sqrt(HD))
MASKNEG = -30000.0

F16 = mybir.dt.float16
F32 = mybir.dt.float32
AF = mybir.ActivationFunctionType
ALU = mybir.AluOpType

_CACHE = {}


def _emit(ctx, tc, dram, out_ap, repeat=1):
    nc = tc.nc

    const_pool = ctx.enter_context(tc.tile_pool(name="const", bufs=1))
    xt_pool = ctx.enter_context(tc.tile_pool(name="xt", bufs=2))
    qk_pool = ctx.enter_context(tc.tile_pool(name="qk", bufs=3))
    e_pool = ctx.enter_context(tc.tile_pool(name="e", bufs=3))
    z_pool = ctx.enter_context(tc.tile_pool(name="z", bufs=3))
    small_pool = ctx.enter_context(tc.tile_pool(name="small", bufs=3))
    psum_proj = ctx.enter_context(tc.tile_pool(name="pproj", bufs=2, space="PSUM"))
    psum_s = ctx.enter_context(tc.tile_pool(name="pscore", bufs=5, space="PSUM"))
    psum_q = ctx.enter_context(tc.tile_pool(name="pquad", bufs=1, space="PSUM"))

    # constants / parameters
    acat_t = []
    for e in range(4):
        t = const_pool.tile([128, 4 * D], F16, tag=f"acat{e}")
        nc.sync.dma_start(t[:], dram["acat"][128 * e : 128 * (e + 1), :])
        acat_t.append(t)
    qbr_t = const_pool.tile([128, 16], F32, tag="qbr")
    nc.sync.dma_start(qbr_t[:], dram["qbr"][:])
    ones_col = const_pool.tile([1, 128], F16, tag="ones_col")
    nc.gpsimd.memset(ones_col[:], 1.0)

    for b in [b for _ in range(repeat) for b in range(BPC)]:
        # ---- load x^T for this batch element ----
        xt_t = []
        for e in range(4):
            t = xt_pool.tile([128, L], F16, tag=f"xt{e}")
            nc.sync.dma_start(t[:], dram["xt"][b, 128 * e : 128 * (e + 1), :])
            xt_t.append(t)
        mneg_t = small_pool.tile([1, L], F16, tag="mneg")
        nc.sync.dma_start(mneg_t[:], dram["mneg"][b])
        acr_t = small_pool.tile([2, L], F32, tag="acr")
        nc.sync.dma_start(acr_t[:], dram["acr"][b])
        ac_t = small_pool.tile([128, 8], F16, tag="ac")
        nc.sync.dma_start(ac_t[:], dram["acb"][b])
        d_ts = []
        for ic in range(4):
            d_t = z_pool.tile([128, L], F16, tag=f"d{ic}")
            nc.sync.dma_start(d_t[:], dram["dmat"][b, 128 * ic : 128 * (ic + 1), :])
            d_ts.append(d_t)

        # ---- projections: QK^T = Acat^T @ x^T ----
        # chain layout along the 2048 columns: [q_inc | k_inc | q_dec | k_dec]
        qk = []
        for dc in range(16):
            ps = psum_proj.tile([128, L], F32, tag="pproj")
            for e in range(4):
                nc.tensor.matmul(
                    ps[:],
                    acat_t[e][:, 128 * dc : 128 * (dc + 1)],
                    xt_t[e][:],
                    start=(e == 0),
                    stop=(e == 3),
                )
            t = qk_pool.tile([128, L], F16, tag=f"qk{dc}")
            # q chains: add the folded bias on the copy; k chains: bias
            # cancels in softmax, zero bias column makes it a plain copy
            nc.vector.tensor_scalar_add(t[:], ps[:], qbr_t[:, dc : dc + 1])
            qk.append(t)

        # ---- scores, masked exp, normalize, head-sum, +D, square ----
        zsq = []
        for ic in range(4):
            rs = small_pool.tile([128, 8], F32, tag="rs")
            E = []
            for attn in range(2):
                for h in range(NH):
                    qdc = (0 if attn == 0 else 8) + h
                    kdc = (4 if attn == 0 else 12) + h
                    ps = psum_s.tile([128, L], F32, tag="pscore")
                    nc.tensor.matmul(
                        ps[:],
                        qk[qdc][:, 128 * ic : 128 * (ic + 1)],
                        qk[kdc][:],
                        start=True,
                        stop=False,
                    )
                    nc.tensor.matmul(
                        ps[:], ones_col[:], mneg_t[:], start=False, stop=True
                    )
                    g = attn * 4 + h
                    e_t = e_pool.tile([128, L], F16, tag=f"e{g}")
                    nc.scalar.activation(
                        e_t[:],
                        ps[:],
                        AF.Exp,
                        scale=SCALE,
                        accum_out=rs[:, g : g + 1],
                    )
                    E.append(e_t)
            # normalize each head with 4x-mode tensor_scalar as soon as its
            # row sum lands (per-column reciprocal), then fold into the
            # running sum with 2x-mode tensor_tensor add/subtract, seeded
            # with the D tile (the chain absorbs the histogram term).
            w = small_pool.tile([128, 8], F32, tag="w")
            en = []
            for g in range(8):
                nc.vector.reciprocal(w[:, g : g + 1], rs[:, g : g + 1])
                t = e_pool.tile([128, L], F16, tag=f"en{g}")
                nc.vector.tensor_scalar_mul(t[:], E[g][:], w[:, g : g + 1])
                en.append(t)
            acc = d_ts[ic]
            for g in range(8):
                nxt = z_pool.tile([128, L], F16, tag=f"zacc{g % 2}")
                if g < 4:
                    nc.vector.tensor_add(nxt[:], acc[:], en[g][:])
                else:
                    nc.vector.tensor_tensor(
                        nxt[:], acc[:], en[g][:], op=ALU.subtract
                    )
                acc = nxt
            zq = z_pool.tile([128, L], F16, tag="zsq")
            nc.scalar.activation(zq[:], acc[:], AF.Square)
            zsq.append(zq)

        # ---- quadratic forms: rows [a^T W ; c^T W], W = z*z ----
        qf = psum_q.tile([2, L], F32, tag="qf")
        for ic in range(4):
            nc.tensor.matmul(
                qf[:],
                ac_t[:, 2 * ic : 2 * (ic + 1)],
                zsq[ic][:],
                start=(ic == 0),
                stop=(ic == 3),
            )
        # ---- final dots: sum_j (a^T W)_j a_j  and  -sum_j (c^T W)_j c_j ----
        fd = small_pool.tile([2, L], F32, tag="fd")
        red = small_pool.tile([2, 1], F32, tag="red")
        nc.vector.tensor_mul(fd[:], qf[:], acr_t[:])
        nc.vector.tensor_reduce(red[:], fd[:], axis=mybir.AxisListType.X, op=ALU.add)
        nc.sync.dma_start(out_ap[b], red[:])


def _build(repeat=1):
    nc = bacc.Bacc(
        "TRN2",
        target_bir_lowering=False,
        debug=False,
        num_devices=NCORES,
    )
    dram = {
        "acat": nc.dram_tensor("acat", [D, 4 * D], F16, kind="ExternalInput").ap(),
        "qbr": nc.dram_tensor("qbr", [128, 16], F32, kind="ExternalInput").ap(),
        "xt": nc.dram_tensor("xt", [BPC, D, L], F16, kind="ExternalInput").ap(),
        "mneg": nc.dram_tensor("mneg", [BPC, 1, L], F16, kind="ExternalInput").ap(),
        "dmat": nc.dram_tensor("dmat", [BPC, L, L], F16, kind="ExternalInput").ap(),
        "acb": nc.dram_tensor("acb", [BPC, 128, 8], F16, kind="ExternalInput").ap(),
        "acr": nc.dram_tensor("acr", [BPC, 2, L], F32, kind="ExternalInput").ap(),
    }
    out_ap = nc.dram_tensor("out", [BPC, 2], F32, kind="ExternalOutput").ap()
    with tile.TileContext(nc) as tc, ExitStack() as ctx:
        _emit(ctx, tc, dram, out_ap, repeat=repeat)
    nc.compile()
    return nc


def get_nc(repeat=1):
    key = f"nc{repeat}"
    if key not in _CACHE:
        _CACHE[key] = _build(repeat=repeat)
    return _CACHE[key]


def _fold(cw, cb, W, bb):
    # q = (x @ cw.T + cb) @ W.T + bb  ==  x @ A + bias
    A = (W.astype(np.float64) @ cw.astype(np.float64)).T
    bias = cb.astype(np.float64) @ W.astype(np.float64).T + bb
    return A.astype(np.float32), bias.astype(np.float32)


def prepare_in_maps(inputs):
    me = np.asarray(inputs["molecule_embedding"], np.float32)  # [L, B, D]
    src_bond = np.asarray(inputs["src_bond"]).astype(np.int64)  # [B, L, 6]
    tgt_bond = np.asarray(inputs["tgt_bond"]).astype(np.int64)
    src_mask = np.asarray(inputs["src_mask"]).astype(bool)  # [B, L]
    tgt_mask = np.asarray(inputs["tgt_mask"]).astype(bool)

    A_qi, b_qi = _fold(inputs["inc_q_w"], inputs["inc_q_b"], inputs["inc_Wq"], inputs["inc_bq"])
    A_ki, _ = _fold(inputs["inc_k_w"], inputs["inc_k_b"], inputs["inc_Wk"], inputs["inc_bk"])
    A_qd, b_qd = _fold(inputs["dec_q_w"], inputs["dec_q_b"], inputs["dec_Wq"], inputs["dec_bq"])
    A_kd, _ = _fold(inputs["dec_k_w"], inputs["dec_k_b"], inputs["dec_Wk"], inputs["dec_bk"])
    acat = np.concatenate([A_qi, A_ki, A_qd, A_kd], axis=1).astype(np.float16)
    # [128, 16] fp32: bias for d-chunk dc lives in column dc (zeros for k chains)
    qbr = (
        np.concatenate([b_qi, np.zeros(D, np.float32), b_qd, np.zeros(D, np.float32)])
        .astype(np.float32)
        .reshape(16, 128)
        .T.copy()
    )

    a = 1.0 - src_mask.astype(np.float32)  # pad
    t = tgt_mask.astype(np.float32)
    g = 1.0 - t
    c = a * t

    # bond histograms -> D = H_src - (g_i g_j) H_tgt   (small exact integers)
    bi = np.arange(B)[:, None, None]
    li = np.arange(L)[None, :, None]
    H_s = np.zeros((B, L, L), np.float32)
    np.add.at(H_s, (bi, li, src_bond), 1.0)
    H_t = np.zeros((B, L, L), np.float32)
    np.add.at(H_t, (bi, li, tgt_bond), 1.0)
    Dm = (H_s - g[:, :, None] * g[:, None, :] * H_t).astype(np.float16)

    mneg = (MASKNEG * src_mask.astype(np.float32)).astype(np.float16)[:, None, :]
    # [B, 128, 8]: row p, cols [2*ic, 2*ic+1] = (a, c) at token ic*128+p
    acb = (
        np.stack([a, c], axis=-1)  # [B, L, 2]
        .reshape(B, 4, 128, 2)
        .transpose(0, 2, 1, 3)
        .reshape(B, 128, 8)
        .astype(np.float16)
    )
    acr = np.stack([a, -c], axis=1).astype(np.float32)  # [B, 2, L]
    xt = np.ascontiguousarray(me.transpose(1, 2, 0)).astype(np.float16)  # [B, D, L]

    in_maps = []
    for cid in range(NCORES):
        sl = slice(cid * BPC, (cid + 1) * BPC)
        in_maps.append(
            {
                "acat": acat,
                "qbr": qbr,
                "xt": np.ascontiguousarray(xt[sl]),
                "mneg": np.ascontiguousarray(mneg[sl]),
                "dmat": np.ascontiguousarray(Dm[sl]),
                "acb": np.ascontiguousarray(acb[sl]),
                "acr": np.ascontiguousarray(acr[sl]),
            }
        )
    return in_maps


def finish(results):
    outp = np.concatenate([r["out"] for r in results], axis=0)  # [B, 2]
    return (outp[:, 0] + outp[:, 1]).astype(np.float32)


def kernel(**inputs):
    in_maps = prepare_in_maps(inputs)
    nc = get_nc()
    res = run_bass_kernel_spmd(nc, in_maps, core_ids=list(range(NCORES)))
    return finish(res.results)


if __name__ == "__main__":
    rng = np.random.default_rng(0)
    demo = {"molecule_embedding": rng.standard_normal((L, B, D), dtype=np.float32)}
    print("kernel module loaded OK")
